# revision 6
# baseline (speedup 1.0000x reference)
"""Trainium2 Bass kernel for nn_LoraAttention.

Math (reference): qkv = x@W_qkv.T; lora full proj ql/vl = split(x@W_lora.T + b_lora)
(K-part discarded); low-rank dq = (x@A_q.T)@B_q.T*1/8 (same for v); softmax
attention over H=16 heads, D=64; out = attn_cat@W_out.T + b_out.

Host-side algebra folds every LoRA term into the projection weights:
  Wq_eff = W_qkv[q] + W_lora[q] + (B_q@A_q)/8      (q bias b_lora[q] kept)
  Wk_eff = W_qkv[k]                                 (no bias)
  Wv_eff = W_qkv[v] + W_lora[v] + (B_v@A_v)/8
  v bias b_lora[v] commutes through softmax -> folded into host-side output
  bias: b_eff = b_out + W_out @ b_lora[v].

Sharding: 8 cores = 4 batches x 2 head-groups (8 heads each).  Each core
projects QKV for its heads, does attention, and computes a partial output
projection over its 512 concat dims; host sums the two partials per batch.

Device schedule (engine balance: PE ~330us of bf16 matmul columns, ScalarE
~293us of exp -> PE-bound; keep PE saturated and exp starting ASAP):
  - DMA loads interleaved (wqk chunk then x chunk) so the first projection
    group can run at ~2us.
  - K/Q projection emitted per-512-token group; the first S^T + exp fire
    after just two groups.  V projection per key-chunk inside (t0, nq0);
    next pair's K/Q groups interleave across the current pair's iterations.
  - attention per (pair, nq, mq): row-packed concurrent S^T pair (K=64
    tile_position packing), exp on ScalarE from PSUM (scale 1/8, bf16 out),
    P@[V|1] accumulation with denominator in row 64.
  - normalization per (pair, nq): denominator rows -> SBUF -> one DMA to
    partition 0, bf16 reciprocal, two bf16 K=1 ones-matmul broadcasts (fast;
    fp32 moving operands stream at half rate), DVE multiply, DMA-pack.
  - output projection for chunk nq emitted inside pair 3's nq loop so it
    overlaps the remaining attention instead of trailing the kernel.
"""

import numpy as np
import ml_dtypes

import concourse.bacc as bacc
import concourse.tile as tile
from concourse import mybir
from concourse.bass_utils import run_bass_kernel_spmd

B, N, C = 4, 2048, 1024
H, D = 16, 64
LORA_SCALE = 1.0 / 8.0
ATTN_SCALE = float(D) ** -0.5  # 0.125

f32 = mybir.dt.float32
bf16 = mybir.dt.bfloat16
BF = ml_dtypes.bfloat16

NQ = 4           # token chunks of 512 for moving operands
MQ = 16          # key/token chunks of 128 for S^T partition dim
KC = 8           # contraction chunks of 128 over C
PAIRS = 4        # head pairs per core (8 local heads)

_cache: dict = {}


def _build_program():
    nc = bacc.Bacc("TRN2", target_bir_lowering=False, debug=False, num_devices=8)

    xT_d = nc.dram_tensor("xT", [C, N], bf16, kind="ExternalInput").ap()
    wqk_d = nc.dram_tensor("wqk", [C, 1024], bf16, kind="ExternalInput").ap()
    wv_d = nc.dram_tensor("wv", [C, 512], bf16, kind="ExternalInput").ap()
    wo_d = nc.dram_tensor("wo", [512, C], bf16, kind="ExternalInput").ap()
    bq_d = nc.dram_tensor("bq", [128, 4], f32, kind="ExternalInput").ap()
    outT_d = nc.dram_tensor("outT", [C, N], f32, kind="ExternalOutput").ap()

    EXP = mybir.ActivationFunctionType.Exp

    with tile.TileContext(nc) as tc:
        with (
            tc.tile_pool(name="win", bufs=1) as win,        # weights + x + consts
            tc.tile_pool(name="kq", bufs=1) as kqp,         # K/Q bf16 tiles
            tc.tile_pool(name="vp", bufs=1) as vp,          # [V|1] tiles
            tc.tile_pool(name="pex", bufs=6) as pex,        # exp outputs
            tc.tile_pool(name="acat", bufs=1) as acatp,     # normalized attn (d, nq)
            tc.tile_pool(name="scr", bufs=2) as scr,        # small scratch
            tc.tile_pool(name="osb", bufs=3) as osbp,       # out eviction
            tc.tile_pool(name="pp", bufs=2, space="PSUM") as pp,    # proj/rb/out
            tc.tile_pool(name="sp", bufs=2, space="PSUM") as spp,   # S^T scores
            tc.tile_pool(name="ap", bufs=1, space="PSUM") as app,   # PV accum
        ):
            # ---- loads: wqk chunk then x chunk, so the first projection
            # group's accumulation chain can follow the DMA stream ----
            xt, wqk = [], []
            for kc in range(KC):
                tw = win.tile([128, 1024], bf16, tag=f"wqk{kc}")
                nc.sync.dma_start(tw[:], wqk_d[kc * 128:(kc + 1) * 128, :])
                wqk.append(tw)
                t = win.tile([128, N], bf16, tag=f"xt{kc}")
                nc.sync.dma_start(t[:], xT_d[kc * 128:(kc + 1) * 128, :])
                xt.append(t)
            wv = []
            for kc in range(KC):
                t = win.tile([128, 512], bf16, tag=f"wv{kc}")
                nc.sync.dma_start(t[:], wv_d[kc * 128:(kc + 1) * 128, :])
                wv.append(t)
            wo = []
            for dc in range(4):
                t = win.tile([128, 1024], bf16, tag=f"wo{dc}")
                nc.sync.dma_start(t[:], wo_d[dc * 128:(dc + 1) * 128, :])
                wo.append(t)
            bqt = win.tile([128, 4], f32, tag="bq")
            nc.sync.dma_start(bqt[:], bq_d[:])
            ones64 = win.tile([1, 64], bf16, tag="ones64")
            nc.vector.memset(ones64[:], 1.0)

            acat = [[None] * PAIRS for _ in range(NQ)]

            def kq_group(t, kt, qt, kind, nq):
                """kind 0 -> K columns nq*512.., kind 1 -> Q columns."""
                ps = pp.tile([128, 512], f32, tag="pp")
                off = (512 if kind == 0 else 0) + t * 128
                for kc in range(KC):
                    nc.tensor.matmul(
                        ps[:],
                        wqk[kc][:, off:off + 128],
                        xt[kc][:, nq * 512:(nq + 1) * 512],
                        start=(kc == 0), stop=(kc == KC - 1),
                    )
                if kind == 0:
                    nc.vector.tensor_copy(kt[:, nq * 512:(nq + 1) * 512], ps[:])
                else:
                    nc.vector.tensor_scalar_add(
                        qt[:, nq * 512:(nq + 1) * 512], ps[:], bqt[:, t:t + 1]
                    )

            def v_proj(mq):
                vt = vp.tile([128, 8, 65], bf16, tag=f"v{mq}")
                nc.vector.memset(vt[:, :, 64:65], 1.0)
                ps = pp.tile([128, 512], f32, tag="pp")
                for kc in range(KC):
                    nc.tensor.matmul(
                        ps[:], xt[kc][:, mq * 128:(mq + 1) * 128], wv[kc][:],
                        start=(kc == 0), stop=(kc == KC - 1),
                    )
                nc.vector.tensor_copy(
                    vt[:, :, 0:64], ps[:].rearrange("p (h e) -> p h e", h=8)
                )
                return vt

            vts = [None] * MQ
            kq_tiles = {}

            def alloc_kq(t):
                kt = kqp.tile([128, N], bf16, tag=f"k{t}")
                qt = kqp.tile([128, N], bf16, tag=f"q{t}")
                kq_tiles[t] = (kt, qt)
                return kt, qt

            def attn_iter(t, kt, qt, atA, atB, nq, mq):
                sp = spp.tile([128, 1024], f32, tag="sp")
                nc.tensor.matmul(
                    sp[:, 0:512],
                    kt[0:64, mq * 128:(mq + 1) * 128],
                    qt[0:64, nq * 512:(nq + 1) * 512],
                    start=True, stop=True, tile_position=(0, 0),
                )
                nc.tensor.matmul(
                    sp[:, 512:1024],
                    kt[64:128, mq * 128:(mq + 1) * 128],
                    qt[64:128, nq * 512:(nq + 1) * 512],
                    start=True, stop=True, tile_position=(64, 0),
                )
                pe = pex.tile([128, 1024], bf16, tag="pe")
                nc.scalar.activation(pe[:], sp[:], EXP, scale=ATTN_SCALE)
                nc.tensor.matmul(
                    atA[:], vts[mq][:, 2 * t, :], pe[:, 0:512],
                    start=(mq == 0), stop=(mq == MQ - 1),
                )
                nc.tensor.matmul(
                    atB[:], vts[mq][:, 2 * t + 1, :], pe[:, 512:1024],
                    start=(mq == 0), stop=(mq == MQ - 1),
                )

            def norm(t, nq, atA, atB):
                """acat[nq][t] <- atX[0:64] * (1 / atX[64]) for both heads."""
                # ell rows: 64 = raw denominators (psum partition), 0 = DMA'd
                # to partition 0 for the K=1 broadcast matmul, 1 = reciprocal
                ell = scr.tile([65, 1024], f32, tag="ell")
                nc.vector.tensor_copy(ell[64:65, 0:512], atA[64:65, :])
                nc.vector.tensor_copy(ell[64:65, 512:1024], atB[64:65, :])
                nc.sync.dma_start(ell[0:1, :], ell[64:65, :])
                rrf = scr.tile([1, 1024], f32, tag="rrf")
                nc.vector.reciprocal_approx_fast(rrf[0:1, :], ell[0:1, :])
                rrb = scr.tile([1, 1024], bf16, tag="rrb")
                nc.vector.tensor_copy(rrb[0:1, :], rrf[0:1, :])
                ac = acatp.tile([128, 512], bf16, tag=f"ac{nq}_{t}")
                acat[nq][t] = ac
                for half, at in ((0, atA), (1, atB)):
                    rb = pp.tile([128, 512], f32, tag="pp")
                    nc.tensor.matmul(
                        rb[0:64, :], ones64[0:1, :],
                        rrb[0:1, half * 512:(half + 1) * 512],
                        start=True, stop=True,
                    )
                    ar = scr.tile([64, 512], bf16, tag="ar")
                    nc.vector.tensor_copy(ar[:], at[0:64, :])
                    acn = scr.tile([64, 512], bf16, tag="acn")
                    nc.vector.tensor_mul(acn[:], ar[:], rb[0:64, :])
                    nc.sync.dma_start(
                        ac[half * 64:(half + 1) * 64, :], acn[:]
                    )

            def out_proj(nq):
                for cc in range(8):
                    ps = pp.tile([128, 512], f32, tag="pp")
                    for dc in range(4):
                        nc.tensor.matmul(
                            ps[:],
                            wo[dc][:, cc * 128:(cc + 1) * 128],
                            acat[nq][dc][:],
                            start=(dc == 0), stop=(dc == 3),
                        )
                    ob = osbp.tile([128, 512], f32, tag="ob")
                    nc.vector.tensor_copy(ob[:], ps[:])
                    nc.sync.dma_start(
                        outT_d[cc * 128:(cc + 1) * 128, nq * 512:(nq + 1) * 512],
                        ob[:],
                    )

            # ---- pair 0 prologue: just enough projection for mq 0..3 ----
            kt0, qt0 = alloc_kq(0)
            kq_group(0, kt0, qt0, 0, 0)
            kq_group(0, kt0, qt0, 1, 0)

            # interleave points for next pair's 8 projection groups
            ILV = {t: [(1, 3), (1, 8), (1, 13), (2, 3), (2, 8), (2, 13),
                       (3, 4), (3, 10)] if t == 0 else
                      [(0, 5), (0, 11), (1, 5), (1, 11), (2, 5), (2, 11),
                       (3, 5), (3, 11)] for t in range(PAIRS)}

            for t in range(PAIRS):
                kt, qt = kq_tiles.pop(t)
                if t + 1 < PAIRS:
                    ktn, qtn = alloc_kq(t + 1)
                    jobs = [(0, j) for j in range(NQ)] + [(1, j) for j in range(NQ)]
                else:
                    jobs = []
                points = ILV[t]
                for nq in range(NQ):
                    if t == 0 and nq > 0:
                        kq_group(0, kt, qt, 1, nq)
                    atA = app.tile([65, 512], f32, tag="atA")
                    atB = app.tile([65, 512], f32, tag="atB")
                    for mq in range(MQ):
                        if t == 0 and nq == 0:
                            if mq % 4 == 0 and mq > 0:
                                kq_group(0, kt, qt, 0, mq // 4)
                            vts[mq] = v_proj(mq)
                        if jobs and (nq, mq) in points:
                            kind, j = jobs.pop(0)
                            kq_group(t + 1, ktn, qtn, kind, j)
                        attn_iter(t, kt, qt, atA, atB, nq, mq)
                    norm(t, nq, atA, atB)
                    if t == PAIRS - 1:
                        out_proj(nq)

    nc.compile()
    return nc


def _get_program():
    if "nc" not in _cache:
        _cache["nc"] = _build_program()
    return _cache["nc"]


def _prep_in_maps(x, W_qkv, W_lora, b_lora, A_q, B_q, A_v, B_v, W_out):
    HD = H * D  # 1024
    Wq = W_qkv[0:HD] + W_lora[0:HD] + LORA_SCALE * (B_q @ A_q)
    Wk = W_qkv[HD:2 * HD]
    Wv = W_qkv[2 * HD:3 * HD] + W_lora[2 * HD:3 * HD] + LORA_SCALE * (B_v @ A_v)
    bq = b_lora[0:HD]

    xT = [np.ascontiguousarray(x[b].T).astype(BF) for b in range(B)]
    in_maps = []
    for c in range(8):
        b, hg = divmod(c, 2)
        sel = slice(hg * 512, (hg + 1) * 512)
        wqk_c = np.ascontiguousarray(
            np.concatenate([Wq[sel], Wk[sel]], axis=0).T
        ).astype(BF)
        wv_c = np.ascontiguousarray(Wv[sel].T).astype(BF)
        wo_c = np.ascontiguousarray(W_out[:, sel].T).astype(BF)
        bq_c = np.ascontiguousarray(bq[sel].reshape(4, 128).T).astype(np.float32)
        in_maps.append({
            "xT": xT[b], "wqk": wqk_c, "wv": wv_c, "wo": wo_c, "bq": bq_c,
        })
    return in_maps


def kernel(x, W_qkv, W_lora, b_lora, A_q, B_q, A_v, B_v, W_out, b_out):
    x = np.asarray(x, np.float32)
    W_qkv = np.asarray(W_qkv, np.float32)
    W_lora = np.asarray(W_lora, np.float32)
    b_lora = np.asarray(b_lora, np.float32)
    A_q = np.asarray(A_q, np.float32)
    B_q = np.asarray(B_q, np.float32)
    A_v = np.asarray(A_v, np.float32)
    B_v = np.asarray(B_v, np.float32)
    W_out = np.asarray(W_out, np.float32)
    b_out = np.asarray(b_out, np.float32)

    in_maps = _prep_in_maps(x, W_qkv, W_lora, b_lora, A_q, B_q, A_v, B_v, W_out)
    b_eff = b_out + W_out @ b_lora[2 * H * D:3 * H * D]

    nc = _get_program()
    res = run_bass_kernel_spmd(nc, in_maps, list(range(8)))

    out = np.empty((B, N, C), np.float32)
    for b in range(B):
        acc = res.results[2 * b]["outT"] + res.results[2 * b + 1]["outT"]
        acc += b_eff[:, None]
        out[b] = acc.T
    return out


# revision 9
# speedup vs baseline: 1.0209x; 1.0209x over previous
"""Trainium2 Bass kernel for nn_LoraAttention.

Math (reference): qkv = x@W_qkv.T; lora full proj ql/vl = split(x@W_lora.T + b_lora)
(K-part discarded); low-rank dq = (x@A_q.T)@B_q.T*1/8 (same for v); softmax
attention over H=16 heads, D=64; out = attn_cat@W_out.T + b_out.

Host-side algebra folds every LoRA term into the projection weights:
  Wq_eff = W_qkv[q] + W_lora[q] + (B_q@A_q)/8      (q bias b_lora[q] kept)
  Wk_eff = W_qkv[k]                                 (no bias)
  Wv_eff = W_qkv[v] + W_lora[v] + (B_v@A_v)/8
  v bias b_lora[v] commutes through softmax -> folded into host-side output
  bias: b_eff = b_out + W_out @ b_lora[v].

Sharding: 8 cores = 4 batches x 2 head-groups (8 heads each).  Each core
projects QKV for its heads, does attention, and computes a partial output
projection over its 512 concat dims; host sums the two partials per batch.

Device schedule (engine balance: PE ~330us of bf16 matmul columns, ScalarE
~293us of exp -> PE-bound; keep PE saturated and exp starting ASAP):
  - DMA loads interleaved (wqk chunk then x chunk) so the first projection
    group can run at ~2us.
  - K/Q projection emitted per-512-token group; the first S^T + exp fire
    after just two groups.  V projection per key-chunk inside (t0, nq0);
    next pair's K/Q groups interleave across the current pair's iterations.
  - attention per (pair, nq, mq): row-packed concurrent S^T pair (K=64
    tile_position packing), exp on ScalarE from PSUM (scale 1/8, bf16 out),
    P@[V|1] accumulation with denominator in row 64.
  - normalization per (pair, nq): denominator rows -> SBUF -> one DMA to
    partition 0, bf16 reciprocal, two bf16 K=1 ones-matmul broadcasts (fast;
    fp32 moving operands stream at half rate), DVE multiply, DMA-pack.
  - output projection for chunk nq emitted inside pair 3's nq loop so it
    overlaps the remaining attention instead of trailing the kernel.
"""

import numpy as np
import ml_dtypes

import concourse.bacc as bacc
import concourse.tile as tile
from concourse import mybir
from concourse.bass_utils import run_bass_kernel_spmd

B, N, C = 4, 2048, 1024
H, D = 16, 64
LORA_SCALE = 1.0 / 8.0
ATTN_SCALE = float(D) ** -0.5  # 0.125

f32 = mybir.dt.float32
bf16 = mybir.dt.bfloat16
BF = ml_dtypes.bfloat16

NQ = 4           # token chunks of 512 for moving operands
MQ = 16          # key/token chunks of 128 for S^T partition dim
KC = 8           # contraction chunks of 128 over C
PAIRS = 4        # head pairs per core (8 local heads)

_cache: dict = {}


def _build_program():
    nc = bacc.Bacc("TRN2", target_bir_lowering=False, debug=False, num_devices=8)

    xT_d = nc.dram_tensor("xT", [C, N], bf16, kind="ExternalInput").ap()
    wqk_d = nc.dram_tensor("wqk", [C, 1024], bf16, kind="ExternalInput").ap()
    wv_d = nc.dram_tensor("wv", [C, 512], bf16, kind="ExternalInput").ap()
    wo_d = nc.dram_tensor("wo", [512, C], bf16, kind="ExternalInput").ap()
    bq_d = nc.dram_tensor("bq", [128, 4], f32, kind="ExternalInput").ap()
    outT_d = nc.dram_tensor("outT", [C, N], f32, kind="ExternalOutput").ap()

    EXP = mybir.ActivationFunctionType.Exp

    with tile.TileContext(nc) as tc:
        with (
            tc.tile_pool(name="win", bufs=1) as win,        # weights + x + consts
            tc.tile_pool(name="kq", bufs=1) as kqp,         # K/Q bf16 tiles
            tc.tile_pool(name="vp", bufs=1) as vp,          # [V|1] tiles
            tc.tile_pool(name="pex", bufs=6) as pex,        # exp outputs
            tc.tile_pool(name="acat", bufs=1) as acatp,     # normalized attn (d, nq)
            tc.tile_pool(name="scr", bufs=2) as scr,        # small scratch
            tc.tile_pool(name="osb", bufs=3) as osbp,       # out eviction
            tc.tile_pool(name="pp", bufs=2, space="PSUM") as pp,    # proj/rb/out
            tc.tile_pool(name="sp", bufs=2, space="PSUM") as spp,   # S^T scores
            tc.tile_pool(name="ap", bufs=1, space="PSUM") as app,   # PV accum
        ):
            # ---- loads: wqk chunk then x chunk, so the first projection
            # group's accumulation chain can follow the DMA stream ----
            xt, wqk = [], []
            for kc in range(KC):
                tw = win.tile([128, 1024], bf16, tag=f"wqk{kc}")
                nc.sync.dma_start(tw[:], wqk_d[kc * 128:(kc + 1) * 128, :])
                wqk.append(tw)
                t = win.tile([128, N], bf16, tag=f"xt{kc}")
                nc.sync.dma_start(t[:], xT_d[kc * 128:(kc + 1) * 128, :])
                xt.append(t)
            wv = []
            for kc in range(KC):
                t = win.tile([128, 512], bf16, tag=f"wv{kc}")
                nc.sync.dma_start(t[:], wv_d[kc * 128:(kc + 1) * 128, :])
                wv.append(t)
            wo = []
            for dc in range(4):
                t = win.tile([128, 1024], bf16, tag=f"wo{dc}")
                nc.sync.dma_start(t[:], wo_d[dc * 128:(dc + 1) * 128, :])
                wo.append(t)
            bqt = win.tile([128, 4], f32, tag="bq")
            nc.sync.dma_start(bqt[:], bq_d[:])
            ones64 = win.tile([1, 64], bf16, tag="ones64")
            nc.vector.memset(ones64[:], 1.0)

            acat = [[None] * PAIRS for _ in range(NQ)]

            def kq_group(t, kt, qt, kind, nq):
                """kind 0 -> K columns nq*512.., kind 1 -> Q columns."""
                ps = pp.tile([128, 512], f32, tag="pp")
                off = (512 if kind == 0 else 0) + t * 128
                for kc in range(KC):
                    nc.tensor.matmul(
                        ps[:],
                        wqk[kc][:, off:off + 128],
                        xt[kc][:, nq * 512:(nq + 1) * 512],
                        start=(kc == 0), stop=(kc == KC - 1),
                    )
                if kind == 0:
                    nc.vector.tensor_copy(kt[:, nq * 512:(nq + 1) * 512], ps[:])
                else:
                    nc.vector.tensor_scalar_add(
                        qt[:, nq * 512:(nq + 1) * 512], ps[:], bqt[:, t:t + 1]
                    )

            def v_proj(mq):
                vt = vp.tile([128, 8, 65], bf16, tag=f"v{mq}")
                nc.vector.memset(vt[:, :, 64:65], 1.0)
                ps = pp.tile([128, 512], f32, tag="pp")
                for kc in range(KC):
                    nc.tensor.matmul(
                        ps[:], xt[kc][:, mq * 128:(mq + 1) * 128], wv[kc][:],
                        start=(kc == 0), stop=(kc == KC - 1),
                    )
                nc.vector.tensor_copy(
                    vt[:, :, 0:64], ps[:].rearrange("p (h e) -> p h e", h=8)
                )
                return vt

            vts = [None] * MQ
            kq_tiles = {}

            def alloc_kq(t):
                kt = kqp.tile([128, N], bf16, tag=f"k{t}")
                qt = kqp.tile([128, N], bf16, tag=f"q{t}")
                kq_tiles[t] = (kt, qt)
                return kt, qt

            def attn_iter(t, kt, qt, atA, atB, nq, mq):
                sp = spp.tile([128, 1024], f32, tag="sp")
                nc.tensor.matmul(
                    sp[:, 0:512],
                    kt[0:64, mq * 128:(mq + 1) * 128],
                    qt[0:64, nq * 512:(nq + 1) * 512],
                    start=True, stop=True, tile_position=(0, 0),
                )
                nc.tensor.matmul(
                    sp[:, 512:1024],
                    kt[64:128, mq * 128:(mq + 1) * 128],
                    qt[64:128, nq * 512:(nq + 1) * 512],
                    start=True, stop=True, tile_position=(64, 0),
                )
                pe = pex.tile([128, 1024], bf16, tag="pe")
                nc.scalar.activation(pe[:], sp[:], EXP, scale=ATTN_SCALE)
                nc.tensor.matmul(
                    atA[:], vts[mq][:, 2 * t, :], pe[:, 0:512],
                    start=(mq == 0), stop=(mq == MQ - 1),
                )
                nc.tensor.matmul(
                    atB[:], vts[mq][:, 2 * t + 1, :], pe[:, 512:1024],
                    start=(mq == 0), stop=(mq == MQ - 1),
                )

            def norm_front(t, nq, atA, atB):
                """DVE/DMA part of normalization: free atA/atB fast (ar copies
                first), then build 1/ell in bf16 at partition 0.  No PE
                instructions here — the in-order TensorE queue must not block
                on this serial chain."""
                arA = scr.tile([64, 512], bf16, tag="arA")
                nc.vector.tensor_copy(arA[:], atA[0:64, :])
                arB = scr.tile([64, 512], bf16, tag="arB")
                nc.vector.tensor_copy(arB[:], atB[0:64, :])
                ell = scr.tile([65, 1024], f32, tag="ell")
                nc.vector.tensor_copy(ell[64:65, 0:512], atA[64:65, :])
                nc.vector.tensor_copy(ell[64:65, 512:1024], atB[64:65, :])
                nc.sync.dma_start(ell[0:1, :], ell[64:65, :])
                rrf = scr.tile([1, 1024], f32, tag="rrf")
                nc.vector.reciprocal_approx_fast(rrf[0:1, :], ell[0:1, :])
                rrb = scr.tile([1, 1024], bf16, tag="rrb")
                nc.vector.tensor_copy(rrb[0:1, :], rrf[0:1, :])
                return (t, nq, arA, arB, rrb)

            def norm_back(state):
                """PE broadcast + multiply + pack; emitted a few iterations
                later so rrb is ready when the PE pops these matmuls."""
                t, nq, arA, arB, rrb = state
                ac = acatp.tile([128, 512], bf16, tag=f"ac{nq}_{t}")
                acat[nq][t] = ac
                for half, ar in ((0, arA), (1, arB)):
                    rb = pp.tile([128, 512], f32, tag="pp")
                    nc.tensor.matmul(
                        rb[0:64, :], ones64[0:1, :],
                        rrb[0:1, half * 512:(half + 1) * 512],
                        start=True, stop=True,
                    )
                    acn = scr.tile([64, 512], bf16, tag="acn")
                    nc.vector.tensor_mul(acn[:], ar[:], rb[0:64, :])
                    nc.sync.dma_start(
                        ac[half * 64:(half + 1) * 64, :], acn[:]
                    )

            def out_proj(nq):
                for cc in range(8):
                    ps = pp.tile([128, 512], f32, tag="pp")
                    for dc in range(4):
                        nc.tensor.matmul(
                            ps[:],
                            wo[dc][:, cc * 128:(cc + 1) * 128],
                            acat[nq][dc][:],
                            start=(dc == 0), stop=(dc == 3),
                        )
                    ob = osbp.tile([128, 512], f32, tag="ob")
                    nc.vector.tensor_copy(ob[:], ps[:])
                    nc.sync.dma_start(
                        outT_d[cc * 128:(cc + 1) * 128, nq * 512:(nq + 1) * 512],
                        ob[:],
                    )

            # ---- pair 0 prologue: just enough projection for mq 0..3 ----
            kt0, qt0 = alloc_kq(0)
            kq_group(0, kt0, qt0, 0, 0)
            kq_group(0, kt0, qt0, 1, 0)

            # interleave points for next pair's 8 projection groups
            ILV = {t: [(1, 3), (1, 8), (1, 13), (2, 3), (2, 8), (2, 13),
                       (3, 4), (3, 10)] if t == 0 else
                      [(0, 5), (0, 11), (1, 5), (1, 11), (2, 5), (2, 11),
                       (3, 5), (3, 11)] for t in range(PAIRS)}

            pending = None
            for t in range(PAIRS):
                kt, qt = kq_tiles.pop(t)
                if t + 1 < PAIRS:
                    ktn, qtn = alloc_kq(t + 1)
                    jobs = [(0, j) for j in range(NQ)] + [(1, j) for j in range(NQ)]
                else:
                    jobs = []
                points = ILV[t]
                for nq in range(NQ):
                    if t == 0 and nq > 0:
                        kq_group(0, kt, qt, 1, nq)
                    atA = app.tile([65, 512], f32, tag="atA")
                    atB = app.tile([65, 512], f32, tag="atB")
                    for mq in range(MQ):
                        if mq == 2 and pending is not None:
                            pt, pnq = pending[0], pending[1]
                            norm_back(pending)
                            pending = None
                            if pt == PAIRS - 1:
                                out_proj(pnq)
                        if t == 0 and nq == 0:
                            if mq % 4 == 0 and mq > 0:
                                kq_group(0, kt, qt, 0, mq // 4)
                            vts[mq] = v_proj(mq)
                        if jobs and (nq, mq) in points:
                            kind, j = jobs.pop(0)
                            kq_group(t + 1, ktn, qtn, kind, j)
                        attn_iter(t, kt, qt, atA, atB, nq, mq)
                    pending = norm_front(t, nq, atA, atB)
            norm_back(pending)
            out_proj(NQ - 1)

    nc.compile()
    return nc


def _get_program():
    if "nc" not in _cache:
        _cache["nc"] = _build_program()
    return _cache["nc"]


def _prep_in_maps(x, W_qkv, W_lora, b_lora, A_q, B_q, A_v, B_v, W_out):
    HD = H * D  # 1024
    Wq = W_qkv[0:HD] + W_lora[0:HD] + LORA_SCALE * (B_q @ A_q)
    Wk = W_qkv[HD:2 * HD]
    Wv = W_qkv[2 * HD:3 * HD] + W_lora[2 * HD:3 * HD] + LORA_SCALE * (B_v @ A_v)
    bq = b_lora[0:HD]

    xT = [np.ascontiguousarray(x[b].T).astype(BF) for b in range(B)]
    in_maps = []
    for c in range(8):
        b, hg = divmod(c, 2)
        sel = slice(hg * 512, (hg + 1) * 512)
        wqk_c = np.ascontiguousarray(
            np.concatenate([Wq[sel], Wk[sel]], axis=0).T
        ).astype(BF)
        wv_c = np.ascontiguousarray(Wv[sel].T).astype(BF)
        wo_c = np.ascontiguousarray(W_out[:, sel].T).astype(BF)
        bq_c = np.ascontiguousarray(bq[sel].reshape(4, 128).T).astype(np.float32)
        in_maps.append({
            "xT": xT[b], "wqk": wqk_c, "wv": wv_c, "wo": wo_c, "bq": bq_c,
        })
    return in_maps


def kernel(x, W_qkv, W_lora, b_lora, A_q, B_q, A_v, B_v, W_out, b_out):
    x = np.asarray(x, np.float32)
    W_qkv = np.asarray(W_qkv, np.float32)
    W_lora = np.asarray(W_lora, np.float32)
    b_lora = np.asarray(b_lora, np.float32)
    A_q = np.asarray(A_q, np.float32)
    B_q = np.asarray(B_q, np.float32)
    A_v = np.asarray(A_v, np.float32)
    B_v = np.asarray(B_v, np.float32)
    W_out = np.asarray(W_out, np.float32)
    b_out = np.asarray(b_out, np.float32)

    in_maps = _prep_in_maps(x, W_qkv, W_lora, b_lora, A_q, B_q, A_v, B_v, W_out)
    b_eff = b_out + W_out @ b_lora[2 * H * D:3 * H * D]

    nc = _get_program()
    res = run_bass_kernel_spmd(nc, in_maps, list(range(8)))

    out = np.empty((B, N, C), np.float32)
    for b in range(B):
        acc = res.results[2 * b]["outT"] + res.results[2 * b + 1]["outT"]
        acc += b_eff[:, None]
        out[b] = acc.T
    return out
